# revision 1
# baseline (speedup 1.0000x reference)
"""Trainium2 Bass kernel for nn_ExplicitCircuit (12-qubit batched statevector sim).

Math: the circuit's prefix (H on all qubits + diagonal data-dependent
PhaseShift/IsingZZ) collapses to state[b,s] = (1/64) * exp(i*phi(b,s)) with
phi = x @ M a rank-56 factorized matmul (L^T @ R) directly in the on-chip
layout. The three variational layers are 6 group unitaries (Kron products of
per-qubit RZ*RY*RX, 64x64 complex, host-built from the 108 weights) applied on
the TensorEngine; each application uses the state as the matmul *stationary*
operand which simultaneously transposes the layout, so gates alternate between
the two qubit groups with zero explicit transposes. Ring-CZ entanglers fold
into the gate matrices except two group-crossing edges applied as a +-1 mask.
Measurement of <Z_0> is |amp|^2 reduction with a +-1 partition matmul.

Data parallel: batch 256 -> 8 cores x 32. Weights/constants replicated.

Internal conventions:
  state index s: qubit q <-> bit q of s.  a = s & 63, h = s >> 6.
  per-core batch b = c*16 + j*2 + beta  (c,beta in {0,1}, j in [0,8))
  Layout X: partition p = c*64 + a,    free f = j*128 + beta*64 + h
  Layout Y: partition p = beta*64 + h, free f = j*128 + c*64 + a
  Gate application maps X->Y->X->...
"""
import numpy as np

NQ = 12
NL = 3
DA = 64
DB = 64
BL = 32
NCORES = 8
KROWS = 56

PAIRS = [(i, j) for i in range(NQ) for j in range(i + 1, NQ)]
PAIR_IDX = {p: k for k, p in enumerate(PAIRS)}
AA_PAIRS = [(i, j) for (i, j) in PAIRS if j <= 5]
BB_PAIRS = [(i, j) for (i, j) in PAIRS if i >= 6]

_a = np.arange(DA)
_h = np.arange(DB)
BIT_A = ((_a[None, :] >> np.arange(6)[:, None]) & 1).astype(np.float64)
BIT_H = ((_h[None, :] >> np.arange(6)[:, None]) & 1).astype(np.float64)
CHI_A = 1.0 - 2.0 * BIT_A
CHI_H = 1.0 - 2.0 * BIT_H

TWO_PI = 2.0 * np.pi
INV_TWO_PI = np.float32(1.0 / TWO_PI)
MAGIC = np.float32(1.5 * 2.0 ** 23)  # keeps result in [2^23, 2^24) where ulp=1


def _cody_waite_consts():
    """2*pi = c1 + c2 + c3 with c1, c2 carrying ~12 high mantissa bits each so
    k*c1, k*c2 are exact in fp32 for small integer k."""
    def chop(v):
        f = np.float32(v)
        u = f.view(np.uint32) if np.isscalar(f) else np.float32(f).view(np.uint32)
        u = np.uint32(u & np.uint32(0xFFFFF000))
        return u.view(np.float32)
    c1 = chop(np.float64(TWO_PI))
    c2 = chop(np.float64(TWO_PI) - np.float64(c1))
    c3 = np.float32(np.float64(TWO_PI) - np.float64(c1) - np.float64(c2))
    return float(c1), float(c2), float(c3)


CW1, CW2, CW3 = _cody_waite_consts()


def build_L():
    L = np.zeros((KROWS, 128), np.float64)
    base = np.zeros((28, DA), np.float64)
    for t in range(6):
        base[t] = BIT_A[t]
    for t, (i, ip) in enumerate(AA_PAIRS):
        base[6 + t] = CHI_A[i] * CHI_A[ip]
    for i in range(6):
        base[21 + i] = CHI_A[i]
    base[27] = 1.0
    for c0 in range(2):
        L[c0 * 28:(c0 + 1) * 28, c0 * 64:(c0 + 1) * 64] = base
    return L.astype(np.float32)


def build_R(x_core):
    x = np.asarray(x_core, np.float64)
    R = np.zeros((KROWS, 1024), np.float64)
    coefA = np.zeros((BL, 21), np.float64)
    coefA[:, 0:6] = x[:, 0:6]
    for t, p in enumerate(AA_PAIRS):
        coefA[:, 6 + t] = -0.5 * x[:, 12 + PAIR_IDX[p]]
    hcoef = np.zeros((BL, 6, DB), np.float64)
    for i in range(6):
        for j in range(6, 12):
            hcoef[:, i, :] += (-0.5 * x[:, 12 + PAIR_IDX[(i, j)]])[:, None] * CHI_H[j - 6][None, :]
    phiB = np.zeros((BL, DB), np.float64)
    for j in range(6, 12):
        phiB += x[:, j][:, None] * BIT_H[j - 6][None, :]
    for (i, j) in BB_PAIRS:
        phiB += (-0.5 * x[:, 12 + PAIR_IDX[(i, j)]])[:, None] * (CHI_H[i - 6] * CHI_H[j - 6])[None, :]
    for c0 in range(2):
        for jj in range(8):
            for beta in range(2):
                b = c0 * 16 + jj * 2 + beta
                f0 = jj * 128 + beta * 64
                R[c0 * 28:c0 * 28 + 21, f0:f0 + 64] = coefA[b][:, None]
                R[c0 * 28 + 21:c0 * 28 + 27, f0:f0 + 64] = hcoef[b]
                R[c0 * 28 + 27, f0:f0 + 64] = phiB[b]
    return R.astype(np.float32)


def _rx(t):
    c, s = np.cos(t / 2), np.sin(t / 2)
    return np.array([[c, -1j * s], [-1j * s, c]])


def _ry(t):
    c, s = np.cos(t / 2), np.sin(t / 2)
    return np.array([[c, -s], [s, c]])


def _rz(t):
    return np.diag([np.exp(-0.5j * t), np.exp(0.5j * t)])


def _kron_chain(mats):
    out = np.array([[1.0 + 0j]])
    for m in mats:
        out = np.kron(m, out)
    return out


def _cz_diag(bits):
    d = np.ones(64)
    for i in range(5):
        d *= 1.0 - 2.0 * (bits[i] * bits[i + 1])
    return d


def effective_gates(weights):
    w = np.asarray(weights, np.float64)
    UAs, UBs = [], []
    p = 0
    for _l in range(NL):
        mats = []
        for _q in range(NQ):
            mats.append(_rz(w[p + 2]) @ _ry(w[p + 1]) @ _rx(w[p]))
            p += 3
        UAs.append(_kron_chain(mats[0:6]))
        UBs.append(_kron_chain(mats[6:12]))
    dA, dB = _cz_diag(BIT_A), _cz_diag(BIT_H)
    return [UAs[0] / 64.0, UBs[0],
            UAs[1] * dA[None, :], UBs[1] * dB[None, :],
            UAs[2] * dA[None, :], UBs[2] * dB[None, :]]


def pack_gates(weights):
    G = effective_gates(weights)
    out = np.zeros((128, 18 * 128), np.float32)
    eye2 = np.eye(2)
    for g, Gm in enumerate(G):
        W = np.kron(eye2, Gm.T)
        out[:, (3 * g + 0) * 128:(3 * g + 1) * 128] = W.real
        out[:, (3 * g + 1) * 128:(3 * g + 2) * 128] = W.imag
        out[:, (3 * g + 2) * 128:(3 * g + 3) * 128] = -W.imag
    return out


def cz_mask_X():
    dx = np.ones((DA, DB))
    dx *= 1.0 - 2.0 * np.outer(BIT_A[5], BIT_H[0])
    dx *= 1.0 - 2.0 * np.outer(BIT_A[0], BIT_H[5])
    m = np.zeros((128, 1024), np.float32)
    for c in range(2):
        for jj in range(8):
            for beta in range(2):
                f0 = jj * 128 + beta * 64
                m[c * 64:(c + 1) * 64, f0:f0 + 64] = dx
    return m


def sign_vec():
    sv = np.zeros((128, 2), np.float32)
    s = 1.0 - 2.0 * (np.arange(DA) & 1)
    sv[0:64, 0] = s
    sv[64:128, 1] = s
    return sv


# ----------------------------- device program -----------------------------
# Packed input layout [128, CIN_W] fp32 (single tensor, two DMAs):
#   cols 0:128       rows 0:56   L      (phi lhsT)        -- DMA 1
#   cols 128:1152    rows 0:56   R      (phi rhs)         -- DMA 1
#   cols 1152:4224   all rows    gates: per gate g two packed moving mats
#                                M1=[Wr|Wi], M2=[-Wi|Wr]  (128x256 each) -- DMA 2
#   cols 4224:5248   all rows    CZ cross mask            -- DMA 2
#   cols 5248:5250   all rows    sign vec                 -- DMA 2
OFF_L = 0
OFF_R = 128
OFF_HP = 1152      # pi/2 column (rides DMA 1, full rows: ACT Sin bias)
OFF_L2 = 1154      # L/2pi (rows 0:56) for the magic-round k
OFF_G = 1282       # even -> 8-byte-aligned DMA offsets
OFF_M = 4226
OFF_SV = 5250
CIN_W = 5252
FL2PI = float(np.float32(2.0 * np.pi))

_CACHE = {}


def pack_gates_packed(weights):
    """[128, 3072]: per gate g: M1 = [Wr | Wi], M2 = [-Wi | Wr], each 128x256."""
    G = effective_gates(weights)
    out = np.zeros((128, 6 * 512), np.float32)
    eye2 = np.eye(2)
    for g, Gm in enumerate(G):
        W = np.kron(eye2, Gm.T)
        base = g * 512
        out[:, base + 0:base + 128] = W.real
        out[:, base + 128:base + 256] = W.imag
        out[:, base + 256:base + 384] = -W.imag
        out[:, base + 384:base + 512] = W.real
    return out


def _build_nc(act_copies=False, act_squares=True, split_dma=True, f32r_gates=True):
    """Raw-Bass, hand-scheduled; semaphore tick numbers are COMPUTED from the
    program structure (see ledger below). Every instruction carries at most one
    sync wait (codegen limit); elided waits rest on cross-engine transitivity.

    Gate g (16 matmuls): chunk jj in [0,8): psum[:, jj*256:+256] =
      mm(lhsT=SR chunk, rhs=M1=[Wr|Wi]) start; mm(lhsT=SI chunk, rhs=M2=[-Wi|Wr]) stop
      -> psum cols [jj*256:+128] = new SR chunk, [+128:+256] = new SI chunk.
    PSUM pairs ping-pong; phi lives in pair1[:, 0:1024] until gate 1 overwrites.
    """
    import concourse.bass as bass
    import concourse.mybir as mybir

    fp32 = mybir.dt.float32
    st_dt = mybir.dt.float32r if f32r_gates else fp32
    nc = bass.Bass()

    cin_d = nc.dram_tensor("cin", [128, CIN_W], fp32, kind="ExternalInput")
    if f32r_gates:
        gates_d = nc.dram_tensor("gates", [128, 3072], st_dt, kind="ExternalInput")
    y_d = nc.dram_tensor("y", [2, 16], fp32, kind="ExternalOutput")

    # ---------------- ledger (computed tick numbers) ----------------
    # DVE: init = 4 ops per half (8). Then per gate 0..4 copy pieces.
    # ACT: init = 3 ops per half (6). Then i-piece copies for gates 0,2,4
    #      (if act_copies) and the two squares (if act_squares).
    dv, ac = 16, 10
    dv_after, ac_after = {}, {}      # (g, lo) -> tick level after that piece group
    for g in range(5):
        for lo in (0, 4):
            if g in (1, 3) or not act_copies:
                dv += 2
            else:
                dv += 1
                ac += 1
            dv_after[(g, lo)] = dv
            ac_after[(g, lo)] = ac
    if act_squares:
        ac_squares_done = ac + 4
        dv_sq = dv
    else:
        dv_sq = dv + 4               # copy r, copy i, sq r, sq i on DVE
        ac_squares_done = ac
    dv_add = dv_sq + 1
    dv_reduce = dv_add + 1
    dv_ycopy = dv_reduce + 1
    pe_base = {g: 2 + 16 * g for g in range(6)}
    pe_final = 2 + 96 + 1

    with (
        nc.sbuf_tensor("cin_sb", [128, CIN_W], fp32) as cin,
        nc.sbuf_tensor("gsb", [128, 3072], st_dt) as gsb_t,
        nc.sbuf_tensor("sra", [128, 1024], st_dt) as SRa,
        nc.sbuf_tensor("sia", [128, 1024], st_dt) as SIa,
        nc.sbuf_tensor("srb", [128, 1024], st_dt) as SRb,
        nc.sbuf_tensor("sib", [128, 1024], st_dt) as SIb,
        nc.sbuf_tensor("t1", [128, 1024], fp32) as t1,
        nc.sbuf_tensor("t2", [128, 1024], fp32) as t2,
        nc.sbuf_tensor("rr", [128, 16], fp32) as rr,
        nc.sbuf_tensor("ysb", [2, 16], fp32) as ysb,
        nc.psum_tensor("ps0", [128, 2048], fp32) as ps0,
        nc.psum_tensor("ps1", [128, 2048], fp32) as ps1,
        nc.sbuf_tensor("wz", [128, 64], fp32) as wz,
        nc.sbuf_tensor("ngs", [128, 512], fp32) as ngs,
        nc.semaphore("s_gp") as s_gp,
        nc.semaphore("d1") as d1,
        nc.semaphore("d2") as d2,
        nc.semaphore("s_pe") as s_pe,
        nc.semaphore("s_dve") as s_dve,
        nc.semaphore("s_act") as s_act,
        nc.Block() as block,
    ):
        cin_ap = cin.ap()
        gsb = gsb_t
        l_sb = cin_ap[0:KROWS, OFF_L:OFF_L + 128]
        l2_sb = cin_ap[0:KROWS, OFF_L2:OFF_L2 + 128]
        r_sb = cin_ap[0:KROWS, OFF_R:OFF_R + 1024]
        m_sb = cin_ap[:, OFF_M:OFF_M + 1024]
        s_sb = cin_ap[:, OFF_SV:OFF_SV + 2]
        pairs = [ps0.ap(), ps1.ap()]
        states = [(SRa.ap(), SIa.ap()), (SRb.ap(), SIb.ap())]
        phi = pairs[1][:, 0:1024]

        def r_piece(ps, lo, hi):
            v = ps.rearrange("p (c x) -> p c x", x=256)
            return v[:, lo:hi, 0:128]

        def i_piece(ps, lo, hi):
            v = ps.rearrange("p (c x) -> p c x", x=256)
            return v[:, lo:hi, 128:256]

        def st_piece(sap, lo, hi):
            v = sap.rearrange("p (c x) -> p c x", x=128)
            return v[:, lo:hi, :]

        def msk_piece(lo, hi):
            v = m_sb.rearrange("p (c x) -> p c x", x=128)
            return v[:, lo:hi, :]

        @block.gpsimd
        def _(gpsimd):
            nc.gpsimd.memset(wz.ap(), 0.0).then_inc(s_gp, 1)

        @block.sync
        def _(sync):
            if f32r_gates:
                sync.dma_start(cin_ap[:, 0:OFF_G], cin_d[:, 0:OFF_G]).then_inc(d1, 16)
                sync.dma_start(gsb.ap()[:, 0:1024], gates_d[:, 0:1024]).then_inc(d2, 16)
                sync.dma_start(gsb.ap()[:, 1024:3072], gates_d[:, 1024:3072]).then_inc(d2, 16)
                sync.dma_start(cin_ap[:, OFF_M:CIN_W], cin_d[:, OFF_M:CIN_W]).then_inc(d2, 16)
            elif split_dma:
                sync.dma_start(cin_ap[:, 0:OFF_G], cin_d[:, 0:OFF_G]).then_inc(d1, 16)
                sync.dma_start(cin_ap[:, OFF_G:CIN_W], cin_d[:, OFF_G:CIN_W]).then_inc(d2, 16)
            else:
                sync.dma_start(cin_ap, cin_d[:]).then_inc(d1, 16).then_inc(d2, 16)
            sync.wait_ge(s_dve, dv_ycopy)
            sync.dma_start(y_d[:], ysb.ap()).then_inc(d1, 16)

        @block.tensor
        def _(tensor):
            # HAM warm-up: keep the PE busy during input DMA + init chain so
            # phi and the gates run at full clock. Results are discarded
            # (gate 0 resets the psum region with start=True).
            tensor.wait_ge(s_gp, 1)
            for _ in range(10):
                nc.tensor.matmul(pairs[0][0:64, 0:64], wz.ap(), wz.ap(),
                                 start=True, stop=True)
            tensor.wait_ge(d1, 16)
            for half in range(2):
                nc.tensor.matmul(
                    phi[:, half * 512:(half + 1) * 512],
                    l_sb,
                    r_sb[:, half * 512:(half + 1) * 512],
                    start=True, stop=True,
                ).then_inc(s_pe, 1)
            for _ in range(12):
                nc.tensor.matmul(pairs[0][0:64, 0:64], wz.ap(), wz.ap(),
                                 start=True, stop=True)
            for g in range(6):
                gg = gsb.ap() if f32r_gates else cin_ap
                goff = 0 if f32r_gates else OFF_G
                m1 = gg[:, goff + g * 512:goff + g * 512 + 256]
                m2 = gg[:, goff + g * 512 + 256:goff + g * 512 + 512]
                sr_in, si_in = states[g % 2]
                ps = pairs[g % 2]
                for jj in range(8):
                    if g == 0 and jj % 2 == 0:
                        tensor.wait_ge(s_act, [3, 6, 8, 10][jj // 2])
                        if jj == 0:
                            tensor.wait_ge(d2, 16)
                    if f32r_gates and g == 1 and jj == 0:
                        tensor.wait_ge(d2, 48)
                    if jj in (0, 4):
                        lo = jj
                        if g == 0:
                            pass
                        else:
                            tensor.wait_ge(s_dve, dv_after[(g - 1, lo)])
                            if ac_after[(g - 1, lo)] > (12 if g == 1 else ac_after[(g - 2, 4)] if g >= 2 else 0):
                                tensor.wait_ge(s_act, ac_after[(g - 1, lo)])
                    cr = sr_in[:, jj * 128:(jj + 1) * 128]
                    ci = si_in[:, jj * 128:(jj + 1) * 128]
                    reg = ps[:, jj * 256:(jj + 1) * 256]
                    nc.tensor.matmul(reg, cr, m1, start=True, stop=False).then_inc(s_pe, 1)
                    nc.tensor.matmul(reg, ci, m2, start=False, stop=True).then_inc(s_pe, 1)
            tensor.wait_ge(s_dve, dv_reduce)
            nc.tensor.matmul(pairs[0][0:2, 0:16], s_sb, rr.ap(),
                             start=True, stop=True).then_inc(s_pe, 1)

        @block.vector
        def _(vector):
            kf, rt = t1.ap(), t2.ap()
            for q in range(4):
                c0, c1 = q * 256, (q + 1) * 256
                vector.wait_ge(s_pe, 1 if q < 2 else 2)
                nc.vector.tensor_scalar(kf[:, c0:c1], phi[:, c0:c1], float(INV_TWO_PI),
                                        float(MAGIC), mybir.AluOpType.mult,
                                        mybir.AluOpType.add).then_inc(s_dve, 1)
                nc.vector.tensor_scalar_sub(kf[:, c0:c1], kf[:, c0:c1],
                                            float(MAGIC)).then_inc(s_dve, 1)
                nc.vector.scalar_tensor_tensor(rt[:, c0:c1], kf[:, c0:c1], -FL2PI,
                                               phi[:, c0:c1], mybir.AluOpType.mult,
                                               mybir.AluOpType.add).then_inc(s_dve, 1)
                if q >= 2:
                    ns_ap = ngs.ap()[:, (q - 2) * 256:(q - 1) * 256]
                    nc.vector.tensor_scalar(ns_ap, rt[:, c0:c1], -1.0, None,
                                            mybir.AluOpType.mult).then_inc(s_dve, 1)
                    nc.vector.tensor_tensor(kf[:, c0:c1], rt[:, c0:c1], ns_ap,
                                            mybir.AluOpType.max).then_inc(s_dve, 1)
            for g in range(5):
                ps = pairs[g % 2]
                sr_out, si_out = states[(g + 1) % 2]
                for lo in (0, 4):
                    hi = lo + 4
                    vector.wait_ge(s_pe, pe_base[g] + 8 * (1 + lo // 4))
                    if g in (1, 3):
                        nc.vector.tensor_tensor(st_piece(sr_out, lo, hi),
                                                r_piece(ps, lo, hi), msk_piece(lo, hi),
                                                mybir.AluOpType.mult).then_inc(s_dve, 1)
                        nc.vector.tensor_tensor(st_piece(si_out, lo, hi),
                                                i_piece(ps, lo, hi), msk_piece(lo, hi),
                                                mybir.AluOpType.mult).then_inc(s_dve, 1)
                    elif act_copies:
                        nc.vector.tensor_copy(st_piece(sr_out, lo, hi),
                                              r_piece(ps, lo, hi)).then_inc(s_dve, 1)
                    else:
                        nc.vector.tensor_copy(st_piece(sr_out, lo, hi),
                                              r_piece(ps, lo, hi)).then_inc(s_dve, 1)
                        nc.vector.tensor_copy(st_piece(si_out, lo, hi),
                                              i_piece(ps, lo, hi)).then_inc(s_dve, 1)
            if not act_squares:
                vector.wait_ge(s_pe, pe_base[5] + 16)
                nc.vector.tensor_copy(t1.ap(), r_piece(pairs[1], 0, 8)).then_inc(s_dve, 1)
                nc.vector.tensor_copy(t2.ap(), i_piece(pairs[1], 0, 8)).then_inc(s_dve, 1)
                nc.vector.tensor_tensor(t1.ap(), t1.ap(), t1.ap(),
                                        mybir.AluOpType.mult).then_inc(s_dve, 1)
                nc.vector.tensor_tensor(t2.ap(), t2.ap(), t2.ap(),
                                        mybir.AluOpType.mult).then_inc(s_dve, 1)
            else:
                vector.wait_ge(s_act, ac_squares_done)
            nc.vector.tensor_tensor(t1.ap(), t1.ap(), t2.ap(),
                                    mybir.AluOpType.add).then_inc(s_dve, 1)
            nc.vector.tensor_reduce(rr.ap(), t1.ap().rearrange("p (bb h) -> p bb h", h=64),
                                    mybir.AxisListType.X, mybir.AluOpType.add).then_inc(s_dve, 1)
            vector.wait_ge(s_pe, pe_final)
            nc.vector.tensor_copy(ysb.ap(), pairs[0][0:2, 0:16]).then_inc(s_dve, 1)

        @block.scalar
        def _(scalar):
            dv_q = {0: 3, 1: 6, 2: 11, 3: 16}
            for q in range(4):
                c0, c1 = q * 256, (q + 1) * 256
                scalar.wait_ge(s_dve, dv_q[q])
                if q < 2:
                    nc.scalar.activation(t1.ap()[:, c0:c1], t2.ap()[:, c0:c1],
                                         mybir.ActivationFunctionType.Abs).then_inc(s_act, 1)
                nc.scalar.activation(SIa.ap()[:, c0:c1], t2.ap()[:, c0:c1],
                                     mybir.ActivationFunctionType.Sin).then_inc(s_act, 1)
                nc.scalar.activation(SRa.ap()[:, c0:c1], t1.ap()[:, c0:c1],
                                     mybir.ActivationFunctionType.Sin,
                                     bias=cin_ap[:, OFF_HP:OFF_HP + 1],
                                     scale=-1.0).then_inc(s_act, 1)
            if act_copies:
                for g in (0, 2, 4):
                    ps = pairs[g % 2]
                    si_out = states[(g + 1) % 2][1]
                    for lo in (0, 4):
                        scalar.wait_ge(s_pe, pe_base[g] + 8 * (1 + lo // 4))
                        nc.scalar.copy(st_piece(si_out, lo, lo + 4),
                                       i_piece(ps, lo, lo + 4)).then_inc(s_act, 1)
            if act_squares:
                scalar.wait_ge(s_pe, pe_base[5] + 8)
                nc.scalar.activation(st_piece(t1.ap(), 0, 4), r_piece(pairs[1], 0, 4),
                                     mybir.ActivationFunctionType.Square).then_inc(s_act, 1)
                nc.scalar.activation(st_piece(t2.ap(), 0, 4), i_piece(pairs[1], 0, 4),
                                     mybir.ActivationFunctionType.Square).then_inc(s_act, 1)
                scalar.wait_ge(s_pe, pe_base[5] + 16)
                nc.scalar.activation(st_piece(t1.ap(), 4, 8), r_piece(pairs[1], 4, 8),
                                     mybir.ActivationFunctionType.Square).then_inc(s_act, 1)
                nc.scalar.activation(st_piece(t2.ap(), 4, 8), i_piece(pairs[1], 4, 8),
                                     mybir.ActivationFunctionType.Square).then_inc(s_act, 1)

    return nc


def _pack_cin(R_core, L, gates, mask, sv):
    cin = np.zeros((128, CIN_W), np.float32)
    cin[0:KROWS, OFF_L:OFF_L + 128] = L
    cin[0:KROWS, OFF_L2:OFF_L2 + 128] = L * np.float32(1.0 / TWO_PI)
    cin[0:KROWS, OFF_R:OFF_R + 1024] = R_core
    cin[:, OFF_HP] = np.float32(np.pi / 2)
    cin[:, OFF_G:OFF_G + 3072] = gates
    cin[:, OFF_M:OFF_M + 1024] = mask
    cin[:, OFF_SV:OFF_SV + 2] = sv
    return cin


def _make_in_maps(x, weights, f32r_gates=True):
    L = build_L()
    mask = cz_mask_X()
    sv = sign_vec()
    gates = pack_gates_packed(weights)
    maps = []
    for i in range(NCORES):
        if f32r_gates:
            m = {"cin": _pack_cin(build_R(x[i * BL:(i + 1) * BL]), L, 0.0, mask, sv),
                 "gates": gates}
        else:
            m = {"cin": _pack_cin(build_R(x[i * BL:(i + 1) * BL]), L, gates, mask, sv)}
        maps.append(m)
    return maps


def kernel(x, weights):
    from concourse.bass_utils import run_bass_kernel_spmd

    x = np.ascontiguousarray(np.asarray(x, np.float32))
    weights = np.asarray(weights, np.float32)
    if "nc" not in _CACHE:
        _CACHE["nc"] = _build_nc()
    nc = _CACHE["nc"]
    in_maps = _make_in_maps(x, weights)
    res = run_bass_kernel_spmd(nc, in_maps, core_ids=list(range(NCORES)))
    out = np.concatenate([res.results[i]["y"].reshape(BL) for i in range(NCORES)])
    return out.astype(np.float32)


def run_traced(x, weights):
    """Run with NTFF tracing enabled; returns BassKernelResults (for test.py)."""
    from concourse.bass_utils import run_bass_kernel_spmd

    x = np.ascontiguousarray(np.asarray(x, np.float32))
    weights = np.asarray(weights, np.float32)
    if "nc" not in _CACHE:
        _CACHE["nc"] = _build_nc()
    return run_bass_kernel_spmd(_CACHE["nc"], _make_in_maps(x, weights),
                                core_ids=list(range(NCORES)), trace=True)



# revision 24
# speedup vs baseline: 1.4105x; 1.4105x over previous
"""Trainium2 Bass kernel for nn_ExplicitCircuit (12-qubit batched statevector sim).

Math: the circuit's prefix (H on all qubits + diagonal data-dependent
PhaseShift/IsingZZ) collapses to state[b,s] = (1/64) * exp(i*phi(b,s)) with
phi a rank-30 factorized matmul computed directly in the on-chip layout.
The three variational layers are 6 group unitaries (Kron products of
per-qubit RZ*RY*RX, 64x64 complex, host-built from the 108 weights) applied on
the TensorEngine; each application uses the state as the matmul *stationary*
operand which simultaneously transposes the layout, so gates alternate between
the two qubit groups with zero explicit transposes. Ring-CZ entanglers fold
into the gate matrices except two group-crossing edges applied as a +-1 mask.
Measurement of <Z_0> is |amp|^2 reduction with a +-1 partition matmul.

Data parallel: batch 256 -> 8 cores x 32. Weights/constants replicated.

Internal conventions:
  state index s: qubit q <-> bit q of s.  a = s & 63, h = s >> 6.
  per-core batch b = c*16 + j*2 + beta  (c,beta in {0,1}, j in [0,8))
  Layout X: partition p = c*64 + a,    free f = j*128 + beta*64 + h
  Layout Y: partition p = beta*64 + h, free f = j*128 + c*64 + a
  Gate application maps X->Y->X->...

v6 schedule vs v1 baseline (32.7us -> 23.2us cost-model):
  - phi pack compressed to K=30 f32r (one 384ns DMA vs 1.8us; f32r matmul
    1 cyc/row vs fp32's 4) -> trig phase starts ~2.5us earlier.
  - the final B-group gate (and its CZ diag) commutes with the Z_0
    measurement and preserves per-(b,a) h-norms -> dropped entirely
    (16 matmuls + one full boundary + a shorter tail).
  - measurement done in Y layout: ACT squares gate-4's psum banks, DVE
    reduces (z, a-even) / (z, a-odd) separately via strided 5-D views,
    sign applied by accumulating +sv/-sv matmuls into one psum region.
  - state between gates stored interleaved chunk-major ([re|im] per
    128-col chunk) so boundary traffic is flat full-bank moves.
  - hw hazard found on the way: ACT and DVE must not read the same PSUM
    bank concurrently (faults the exec unit) -> boundary banks are
    partitioned between engines (DVE: 0,2 / ACT: 1,3 on clean
    boundaries; masked boundaries all-DVE); GPSIMD cannot touch PSUM.
"""
import numpy as np

NQ = 12
NL = 3
BL = 32
NCORES = 8

PAIRS = [(i, j) for i in range(NQ) for j in range(i + 1, NQ)]
PAIR_IDX = {p: k for k, p in enumerate(PAIRS)}

_a = np.arange(64)
_h = np.arange(64)
BIT_A = ((_a[None, :] >> np.arange(6)[:, None]) & 1).astype(np.float64)
BIT_H = ((_h[None, :] >> np.arange(6)[:, None]) & 1).astype(np.float64)
CHI_A = 1.0 - 2.0 * BIT_A
CHI_H = 1.0 - 2.0 * BIT_H

TWO_PI = 2.0 * np.pi
INV_TWO_PI = np.float32(1.0 / TWO_PI)
MAGIC = np.float32(1.5 * 2.0 ** 23)  # keeps result in [2^23, 2^24) where ulp=1
FL2PI = float(np.float32(TWO_PI))

# AA pair coef matrix [15 pairs -> (i, ip)], precomputed a-side patterns
AA_PAIRS = [(i, j) for (i, j) in PAIRS if j <= 5]
BB_PAIRS = [(i, j) for (i, j) in PAIRS if i >= 6]
AB_PAIRS = [(i, j) for (i, j) in PAIRS if i < 6 <= j]


def build_lr30(x_core):
    """lhsT [30, 128], rhs [30, 1024] with phi = lhsT.T @ rhs in layout X.

    Rows 0:16  (K=(j',beta')): a-only terms, host-contracted into
               phiA_c[a, (j,beta)]; rhs is the batch indicator pattern.
    Rows 16:30 (7 per c): [chiA_0..5, 1] x [hcoef rows, phiB row].
    """
    x = np.asarray(x_core, np.float64)  # [32, 78]
    lhsT = np.zeros((30, 128))
    rhs = np.zeros((30, 1024))

    # a-side per-batch coefficient stack: [27ish terms] -> phiA [b, a]
    # phiA[b, a] = sum_i<6 x[b,i] bitA_i(a) + sum_AA -0.5 x chi chi
    phiA = x[:, 0:6] @ BIT_A  # [32, 64]
    for (i, ip) in AA_PAIRS:
        phiA += np.outer(-0.5 * x[:, 12 + PAIR_IDX[(i, ip)]], CHI_A[i] * CHI_A[ip])
    for c in range(2):
        for k in range(16):
            b = c * 16 + k
            lhsT[k, c * 64:(c + 1) * 64] = phiA[b]
    for k in range(16):
        jj, beta = k // 2, k % 2
        rhs[k, jj * 128 + beta * 64: jj * 128 + beta * 64 + 64] = 1.0

    # cross terms: hcoef[b, t, h] = sum_{j>=6} -0.5 x[b, pair(t,j)] chiH_{j-6}
    hcoef = np.zeros((BL, 6, 64))
    for (i, j) in AB_PAIRS:
        hcoef[:, i, :] += np.outer(-0.5 * x[:, 12 + PAIR_IDX[(i, j)]], CHI_H[j - 6])
    phiB = x[:, 6:12] @ BIT_H
    for (i, j) in BB_PAIRS:
        phiB += np.outer(-0.5 * x[:, 12 + PAIR_IDX[(i, j)]], CHI_H[i - 6] * CHI_H[j - 6])

    for c in range(2):
        base = 16 + 7 * c
        for t in range(6):
            lhsT[base + t, c * 64:(c + 1) * 64] = CHI_A[t]
        lhsT[base + 6, c * 64:(c + 1) * 64] = 1.0
        for jj in range(8):
            for beta in range(2):
                b = c * 16 + jj * 2 + beta
                f0 = jj * 128 + beta * 64
                rhs[base + 0:base + 6, f0:f0 + 64] = hcoef[b]
                rhs[base + 6, f0:f0 + 64] = phiB[b]
    out = np.zeros((30, 1152), np.float32)
    out[:, 0:128] = lhsT
    out[:, 128:1152] = rhs
    return out


def _rx(t):
    c, s = np.cos(t / 2), np.sin(t / 2)
    return np.array([[c, -1j * s], [-1j * s, c]])


def _ry(t):
    c, s = np.cos(t / 2), np.sin(t / 2)
    return np.array([[c, -s], [s, c]])


def _rz(t):
    return np.diag([np.exp(-0.5j * t), np.exp(0.5j * t)])


def _kron_chain(mats):
    out = np.array([[1.0 + 0j]])
    for m in mats:
        out = np.kron(m, out)
    return out


def _cz_diag(bits):
    d = np.ones(64)
    for i in range(5):
        d *= 1.0 - 2.0 * (bits[i] * bits[i + 1])
    return d


def effective_gates(weights):
    w = np.asarray(weights, np.float64)
    UAs, UBs = [], []
    p = 0
    for _l in range(NL):
        mats = []
        for _q in range(NQ):
            mats.append(_rz(w[p + 2]) @ _ry(w[p + 1]) @ _rx(w[p]))
            p += 3
        UAs.append(_kron_chain(mats[0:6]))
        UBs.append(_kron_chain(mats[6:12]))
    dA = _cz_diag(BIT_A)
    # Gate 5 (UBs[2] and ring-2's B-chain CZ diag) acts only on the h
    # qubits; it commutes with Z_0 (an a-side observable) and preserves
    # per-(b,a) h-norms, so it drops out of the expectation entirely.
    dB = _cz_diag(BIT_H)
    return [UAs[0] / 64.0, UBs[0],
            UAs[1] * dA[None, :], UBs[1] * dB[None, :],
            UAs[2] * dA[None, :]]


def pack_gates_packed(weights):
    """[128, 2560]: per gate g: M1 = [Wr | Wi], M2 = [-Wi | Wr], each 128x256."""
    G = effective_gates(weights)
    out = np.zeros((128, 5 * 512), np.float32)
    eye2 = np.eye(2)
    for g, Gm in enumerate(G):
        W = np.kron(eye2, Gm.T)
        base = g * 512
        out[:, base + 0:base + 128] = W.real
        out[:, base + 128:base + 256] = W.imag
        out[:, base + 256:base + 384] = -W.imag
        out[:, base + 384:base + 512] = W.real
    return out


def cz_mask_X():
    """Interleaved chunk-major mask [128, 2048]: per chunk jj the 256 cols
    are [re 128 | im 128], both carrying the same +-1 cross-CZ pattern."""
    dx = np.ones((64, 64))
    dx *= 1.0 - 2.0 * np.outer(BIT_A[5], BIT_H[0])
    dx *= 1.0 - 2.0 * np.outer(BIT_A[0], BIT_H[5])
    m = np.zeros((128, 2048), np.float32)
    for c in range(2):
        for jj in range(8):
            for z in range(2):
                for beta in range(2):
                    f0 = jj * 256 + z * 128 + beta * 64
                    m[c * 64:(c + 1) * 64, f0:f0 + 64] = dx
    return m


def sign_vec():
    """[128, 4]: beta-indicator columns (+1) then negated copies (-1)."""
    sv = np.zeros((128, 4), np.float32)
    sv[0:64, 0] = 1.0
    sv[64:128, 1] = 1.0
    sv[:, 2:4] = -sv[:, 0:2]
    return sv


# ----------------------------- device program -----------------------------

_CACHE = {}

N_WARM = 10


def _build_nc():
    import concourse.bass as bass
    import concourse.mybir as mybir

    fp32 = mybir.dt.float32
    f32r = mybir.dt.float32r
    Alu = mybir.AluOpType
    Act = mybir.ActivationFunctionType
    nc = bass.Bass()

    lr_d = nc.dram_tensor("lr", [30, 1152], f32r, kind="ExternalInput")
    gates_d = nc.dram_tensor("gates", [128, 2560], f32r, kind="ExternalInput")
    aux_d = nc.dram_tensor("aux", [128, 2052], fp32, kind="ExternalInput")
    y_d = nc.dram_tensor("y", [2, 16], fp32, kind="ExternalOutput")

    # --- tick ledger (computed below as streams are declared) ---
    # s_pe: phi 1,2; gate g chunk jj second matmul = 2+16g+2(jj+1); final = 99
    def pe_chunk(g, jj):
        return 2 + 16 * g + 2 * (jj + 1)

    from contextlib import ExitStack
    with ExitStack() as stack:
        ent = stack.enter_context
        lr_sb = ent(nc.sbuf_tensor("lr_sb", [30, 1152], f32r))
        gsb = ent(nc.sbuf_tensor("gsb", [128, 2560], f32r))
        aux_sb = ent(nc.sbuf_tensor("aux_sb", [128, 2052], fp32))
        SRa = ent(nc.sbuf_tensor("sra", [128, 1024], f32r))
        SIa = ent(nc.sbuf_tensor("sia", [128, 1024], f32r))
        SX = ent(nc.sbuf_tensor("sx", [128, 2048], f32r))
        SY = ent(nc.sbuf_tensor("sy", [128, 2048], f32r))
        t1 = ent(nc.sbuf_tensor("t1", [128, 1024], fp32))
        t2 = ent(nc.sbuf_tensor("t2", [128, 1024], fp32))
        rr = ent(nc.sbuf_tensor("rr", [128, 16], fp32))
        rr2 = ent(nc.sbuf_tensor("rr2", [128, 32], fp32))
        ysb = ent(nc.sbuf_tensor("ysb", [2, 16], fp32))
        wz = ent(nc.sbuf_tensor("wz", [128, 64], fp32))
        hp = ent(nc.sbuf_tensor("hp", [128, 1], fp32))
        ps0 = ent(nc.psum_tensor("ps0", [128, 2048], fp32))
        ps1 = ent(nc.psum_tensor("ps1", [128, 2048], fp32))
        s_gp = ent(nc.semaphore("s_gp"))
        d1 = ent(nc.semaphore("d1"))
        d2 = ent(nc.semaphore("d2"))
        s_pe = ent(nc.semaphore("s_pe"))
        s_dve = ent(nc.semaphore("s_dve"))
        s_act = ent(nc.semaphore("s_act"))
        block = ent(nc.Block())
        lr_ap = lr_sb.ap()
        g_ap = gsb.ap()
        aux_ap = aux_sb.ap()
        m_sb = aux_ap[:, 0:2048]
        svp = aux_ap[:, 2048:2050]
        svn = aux_ap[:, 2050:2052]
        pairs = [ps0.ap(), ps1.ap()]
        phi = pairs[1][:, 0:1024]
        sx, sy = SX.ap(), SY.ap()
        t1a, t2a = t1.ap(), t2.ap()

        def q(ap_, k):  # 256-col quarter view
            return ap_[:, k * 256:(k + 1) * 256]

        def r_piece(ps, lo, hi):
            v = ps.rearrange("p (c x) -> p c x", x=256)
            return v[:, lo:hi, 0:128]

        def i_piece(ps, lo, hi):
            v = ps.rearrange("p (c x) -> p c x", x=256)
            return v[:, lo:hi, 128:256]

        def st_piece(sap, lo, hi):
            v = sap.rearrange("p (c x) -> p c x", x=128)
            return v[:, lo:hi, :]

        def msk_piece(lo, hi):
            v = m_sb.rearrange("p (c x) -> p c x", x=128)
            return v[:, lo:hi, :]

        # ------------------------ tick ledger -------------------------
        # HW hazard: ACT and DVE must never read the same PSUM bank
        # concurrently -> boundary traffic is partitioned by 512-col bank
        # (quarter), with the state kept interleaved chunk-major
        # ([re|im] per 128-col chunk) so each boundary piece is one flat
        # full-bank copy.  Gate g>=1 stationary chunks slice SX/SY.
        # s_pe:  phi=1,2; gate g (0..4) chunk jj -> pe_chunk(g,jj)=2+16g+2(jj+1)
        #        (max 82); final mm=83.
        # s_dve: trig q0:1-3 q1:4-6 q2:7-10 q3:11-14
        #        b0 q0=15 q2=16 | b1 mults q0..3 = 17-20
        #        b2 q0=21 q2=22 | b3 mults q0..3 = 23-26
        #        tail red_e/o q0..3 = 27-34, ysb=35
        # s_act: trig abs0=1,i0=2,r0=3,abs1=4,i1=5,r1=6,i2=7,r2=8,i3=9,r3=10
        #        b0 q1=11 q3=12 | b2 q1=13 q3=14
        #        tail sq q0=15 q1=16 q2=17 q3c6=18 q3c7=19
        # s_gp:  wz=1, hp=2
        # tail sq slabs: q0->t1[0:512] q1->t1[512:] q2->t2[512:] q3->t2[0:512]

        bnd = [(0, sx), (1, sy), (2, sx), (3, sy)]

        @block.gpsimd
        def _(gpsimd):
            nc.gpsimd.memset(wz.ap(), 0.0).then_inc(s_gp, 1)
            nc.gpsimd.memset(hp.ap(), float(np.pi / 2)).then_inc(s_gp, 1)

        @block.sync
        def _(sync):
            sync.dma_start(lr_ap, lr_d[:]).then_inc(d1, 16)
            sync.dma_start(g_ap[:, 0:1024], gates_d[:, 0:1024]).then_inc(d2, 16)
            sync.dma_start(aux_ap, aux_d[:]).then_inc(d2, 16)
            sync.dma_start(g_ap[:, 1024:2560], gates_d[:, 1024:2560]).then_inc(d2, 16)
            sync.wait_ge(s_dve, 35)
            sync.dma_start(y_d[:], ysb.ap()).then_inc(d1, 16)

        @block.tensor
        def _(tensor):
            # warm-up: establish pe_busy_start early so phi/gates run at
            # full clock; results discarded.
            tensor.wait_ge(s_gp, 1)
            for _ in range(N_WARM):
                nc.tensor.matmul(pairs[0][0:64, 0:64], wz.ap(), wz.ap(),
                                 start=True, stop=True)
            tensor.wait_ge(d1, 16)
            for half in range(2):
                nc.tensor.matmul(
                    phi[:, half * 512:(half + 1) * 512],
                    lr_ap[0:30, 0:128],
                    lr_ap[0:30, 128 + half * 512:128 + (half + 1) * 512],
                    start=True, stop=True,
                ).then_inc(s_pe, 1)
            g_waits = {
                (0, 0): [(d2, 16), (s_act, 3)],
                (0, 2): [(s_act, 6)],
                (0, 4): [(s_act, 8)],
                (0, 6): [(s_act, 10)],
                (1, 0): [(s_dve, 15)],
                (1, 2): [(s_act, 11)],
                (1, 4): [(s_dve, 16)],
                (1, 6): [(s_act, 12)],
                (2, 0): [(d2, 48), (s_dve, 17)],
                (2, 2): [(s_dve, 18)],
                (2, 4): [(s_dve, 19)],
                (2, 6): [(s_dve, 20)],
                (3, 0): [(s_dve, 21)],
                (3, 2): [(s_act, 13)],
                (3, 4): [(s_dve, 22)],
                (3, 6): [(s_act, 14)],
                (4, 0): [(s_dve, 23)],
                (4, 2): [(s_dve, 24)],
                (4, 4): [(s_dve, 25)],
                (4, 6): [(s_dve, 26)],
            }
            for g in range(5):
                m1 = g_ap[:, g * 512:g * 512 + 256]
                m2 = g_ap[:, g * 512 + 256:g * 512 + 512]
                ps = pairs[g % 2]
                st_in = None if g == 0 else (sx if g % 2 == 1 else sy)
                for jj in range(8):
                    for sem, tick in g_waits.get((g, jj), []):
                        tensor.wait_ge(sem, tick)
                    if g == 0:
                        cr = SRa.ap()[:, jj * 128:(jj + 1) * 128]
                        ci = SIa.ap()[:, jj * 128:(jj + 1) * 128]
                    else:
                        cr = st_in[:, jj * 256:jj * 256 + 128]
                        ci = st_in[:, jj * 256 + 128:jj * 256 + 256]
                    reg = ps[:, jj * 256:(jj + 1) * 256]
                    nc.tensor.matmul(reg, cr, m1, start=True, stop=False).then_inc(s_pe, 1)
                    nc.tensor.matmul(reg, ci, m2, start=False, stop=True).then_inc(s_pe, 1)
            tensor.wait_ge(s_dve, 34)
            nc.tensor.matmul(pairs[1][0:2, 0:16], svp, rr2.ap()[:, 0:16],
                             start=True, stop=False)
            nc.tensor.matmul(pairs[1][0:2, 0:16], svn, rr2.ap()[:, 16:32],
                             start=False, stop=True).then_inc(s_pe, 1)

        @block.vector
        def _(vector):
            # trig chains: kf(t1) = phi/2pi + MAGIC; kf -= MAGIC;
            # rt(t2) = phi - 2pi*kf; |rt|(t1) = max(-rt, rt) (q2,q3 only).
            for k in range(4):
                if k == 0:
                    vector.wait_ge(s_pe, 1)
                if k == 2:
                    vector.wait_ge(s_pe, 2)
                nc.vector.tensor_scalar(q(t1a, k), q(phi, k), float(INV_TWO_PI),
                                        float(MAGIC), Alu.mult,
                                        Alu.add).then_inc(s_dve, 1)
                nc.vector.tensor_scalar_sub(q(t1a, k), q(t1a, k),
                                            float(MAGIC)).then_inc(s_dve, 1)
                nc.vector.scalar_tensor_tensor(q(t2a, k), q(t1a, k), -FL2PI,
                                               q(phi, k), Alu.mult,
                                               Alu.add).then_inc(s_dve, 1)
                if k >= 2:
                    nc.vector.scalar_tensor_tensor(
                        q(t1a, k), q(t2a, k), -1.0, q(t2a, k),
                        Alu.mult, Alu.max).then_inc(s_dve, 1)
            # boundary full-bank moves (DVE owns banks 0 and 2; the masked
            # boundaries' bank 1 too -- ACT owns banks 1,3 on clean, 3 via
            # t2+Pool on masked; never the same bank as DVE at once)
            for g, s_out in bnd:
                ps = pairs[g % 2]
                masked = g in (1, 3)
                if g == 1:
                    vector.wait_ge(d2, 32)
                for k in ((0, 1, 2, 3) if masked else (0, 2)):
                    vector.wait_ge(s_pe, pe_chunk(g, 2 * k + 1))
                    dst = s_out[:, 512 * k:512 * (k + 1)]
                    srcp = ps[:, 512 * k:512 * (k + 1)]
                    if masked:
                        nc.vector.tensor_tensor(
                            dst, srcp, m_sb[:, 512 * k:512 * (k + 1)],
                            Alu.mult).then_inc(s_dve, 1)
                    else:
                        nc.vector.tensor_copy(dst, srcp).then_inc(s_dve, 1)
            # tail: gate-4 output in Y layout: psum chunk cols =
            # jj*256 + z*128 + c*64 + a  (z = re/im, c in free, measured
            # bit = a&1).  |amp|^2 summed over (z, a-even/odd) separately,
            # sign applied by one subtract.
            slab = [t1a[:, 0:512], t1a[:, 512:1024], t2a[:, 512:1024], t2a[:, 0:512]]
            for k in range(4):
                v6 = slab[k].rearrange("p (jj z c ae two) -> p jj c z ae two",
                                       z=2, c=2, ae=32, two=2)
                vector.wait_ge(s_act, 15 + k)
                for par in (0, 1):
                    nc.vector.tensor_reduce(
                        rr2.ap()[:, 16 * par + 4 * k:16 * par + 4 * k + 4]
                        .rearrange("p (jj c) -> p jj c", c=2),
                        v6[:, :, :, :, :, par:par + 1].squeeze(-1),
                        mybir.AxisListType.XY, Alu.add).then_inc(s_dve, 1)  # 27-34
            vector.wait_ge(s_pe, 83)
            nc.vector.tensor_copy(ysb.ap(), pairs[1][0:2, 0:16]).then_inc(s_dve, 1)  # 35

        @block.scalar
        def _(scalar):
            scalar.wait_ge(s_gp, 2)
            # trig sins; |rt| on ACT for q0,q1
            for k in range(4):
                if k < 2:
                    scalar.wait_ge(s_dve, 3 * k + 3)
                    nc.scalar.activation(q(t1a, k), q(t2a, k),
                                         Act.Abs).then_inc(s_act, 1)
                elif k == 2:
                    scalar.wait_ge(s_dve, 10)
                else:
                    scalar.wait_ge(s_dve, 14)
                nc.scalar.activation(q(SIa.ap(), k), q(t2a, k),
                                     Act.Sin).then_inc(s_act, 1)
                nc.scalar.activation(q(SRa.ap(), k), q(t1a, k),
                                     Act.Sin, bias=hp.ap(),
                                     scale=-1.0).then_inc(s_act, 1)
            # boundary banks 1,3 on clean boundaries only
            for g, s_out in bnd:
                ps = pairs[g % 2]
                if g in (1, 3):
                    continue
                for k in (1, 3):
                    scalar.wait_ge(s_pe, pe_chunk(g, 2 * k + 1))
                    nc.scalar.copy(s_out[:, 512 * k:512 * (k + 1)],
                                   ps[:, 512 * k:512 * (k + 1)]).then_inc(s_act, 1)
            # tail |amp|^2 squares: full banks of gate-4 psum
            slab = [t1a[:, 0:512], t1a[:, 512:1024], t2a[:, 512:1024], t2a[:, 0:512]]
            for k in range(4):
                scalar.wait_ge(s_pe, pe_chunk(4, 2 * k + 1))
                nc.scalar.activation(slab[k], pairs[0][:, 512 * k:512 * (k + 1)],
                                     Act.Square).then_inc(s_act, 1)  # 15-18

    return nc


def _make_in_maps(x, weights):
    gates = pack_gates_packed(weights)
    aux = np.zeros((128, 2052), np.float32)
    aux[:, 0:2048] = cz_mask_X()
    aux[:, 2048:2052] = sign_vec()
    maps = []
    for i in range(NCORES):
        maps.append({"lr": build_lr30(x[i * BL:(i + 1) * BL]),
                     "gates": gates, "aux": aux})
    return maps


def kernel(x, weights):
    from concourse.bass_utils import run_bass_kernel_spmd

    x = np.ascontiguousarray(np.asarray(x, np.float32))
    weights = np.asarray(weights, np.float32)
    if "nc" not in _CACHE:
        _CACHE["nc"] = _build_nc()
    nc = _CACHE["nc"]
    in_maps = _make_in_maps(x, weights)
    res = run_bass_kernel_spmd(nc, in_maps, core_ids=list(range(NCORES)))
    # y[beta, jj*2+c] -> batch b = c*16 + jj*2 + beta
    b = np.arange(BL)
    sel = (b & 1, ((b >> 1) & 7) * 2 + (b >> 4))
    out = np.concatenate([res.results[i]["y"][sel] for i in range(NCORES)])
    return out.astype(np.float32)


def run_traced(x, weights):
    """Run with NTFF tracing enabled; returns BassKernelResults (for test.py)."""
    from concourse.bass_utils import run_bass_kernel_spmd

    x = np.ascontiguousarray(np.asarray(x, np.float32))
    weights = np.asarray(weights, np.float32)
    if "nc" not in _CACHE:
        _CACHE["nc"] = _build_nc()
    return run_bass_kernel_spmd(_CACHE["nc"], _make_in_maps(x, weights),
                                core_ids=list(range(NCORES)), trace=True)
